# revision 10
# baseline (speedup 1.0000x reference)
"""Trainium2 Bass kernel for the AdaptiveLIFLayer problem.

LIF scan over T with hard reset, data-parallel over batch across 8 NeuronCores.
  per step: u = (v + x_t)/2 ; s_t = (u >= 1) ; v = u * (u < 1)

Implementation: a custom fused DVE op computes one full timestep per Vector-engine
instruction:  u_t = (select(u_{t-1} < 1, u_{t-1}, 0) + x_t) * 0.5
The state chain lives in SBUF (written in-place over the x tile); spikes are
extracted per chunk with one batched tensor_scalar is_ge.
"""

import os
import sys

import numpy as np

for _p in ("/opt/trn_rl_repo", "/root/.axon_site/_ro/trn_rl_repo"):
    if os.path.isdir(_p) and _p not in sys.path:
        sys.path.insert(0, _p)

# ---- problem constants (hardcoded; kernel.py must be self-contained) ----
B, T, N = 64, 200, 4096
N_CORES = 8
BS = B // N_CORES          # batch per core = 8
K = 16                     # n-chunks per batch row -> BS*K = 128 partitions
E = N // K                 # 256 free elements per partition per step
CHUNK = 25                 # timesteps per DMA chunk
N_CHUNKS = T // CHUNK

_CACHE = {}
LAST_EXEC_NS = None


def _register_lif_op():
    """Register the fused LIF-step custom DVE op (idempotent)."""
    from concourse import dve_ops as dops
    from concourse.dve_spec import Spec, Src0, Src1, C0, C1, Zero, select, lower, _has_src1
    from concourse.dve_uop import DveOpSpec

    if "LIF_STEP_ANT" in dops._SUB_OPCODE_FOR_NAME:
        return dops.CUSTOM_DVE_SPECS and _CACHE["lif_op"]

    def _ref(in0, in1, s0, s1, imm2):
        r = np.where(in0 < s0, in0, np.float32(0.0)).astype(np.float32)
        return ((r + in1) * s1).astype(np.float32)

    spec = Spec(body=(select(Src0 < C0, Src0, Zero) + Src1) * C1, reference=_ref)
    shas = {}
    for ver in ("v3", "v4"):
        uops = lower(spec, ver=ver)
        shas[ver] = DveOpSpec(name="LIF_STEP_ANT", uops=uops, rd1_en=_has_src1(spec)).sha(ver)
    op = dops.DveOp("LIF_STEP_ANT", spec, subdim=False, uops_sha=shas)
    dops.OPS.append(op)
    dops.CUSTOM_DVE_SPECS[op.name] = spec
    dops._SUB_OPCODE_FOR_NAME[op.name] = dops._CUSTOM_DVE_ROW_BASE + len(dops.OPS) - 1
    _CACHE["lif_op"] = op
    return op


def _build():
    """Build + compile the per-core SPMD graph once."""
    if "nc" in _CACHE:
        return _CACHE["nc"]

    from contextlib import ExitStack

    import concourse.bass as bass  # noqa: F401
    import concourse.tile as tile
    from concourse import bacc, mybir

    lif_op = _register_lif_op()

    nc = bacc.Bacc("TRN2", target_bir_lowering=False, debug=False, num_devices=N_CORES)
    f32 = mybir.dt.float32

    # Host pre-transposes each shard to [128, T, E] (partition-major), so every
    # chunk DMA is a clean [128, C*E] pattern with C*E*4-byte contiguous runs.
    x = nc.dram_tensor("x", [128, T, E], f32, kind="ExternalInput")
    out = nc.dram_tensor("out", [128, T, E], f32, kind="ExternalOutput")
    xv = x.ap()
    ov = out.ap()

    with tile.TileContext(nc) as tc, ExitStack() as ctx:
        xpool = ctx.enter_context(tc.tile_pool(name="xu", bufs=3))
        spool = ctx.enter_context(tc.tile_pool(name="sp", bufs=2))
        zpool = ctx.enter_context(tc.tile_pool(name="zz", bufs=1))

        v0 = zpool.tile([128, E], f32)
        nc.vector.memset(v0[:], 0.0)
        prev = v0[:]

        for ci in range(N_CHUNKS):
            xt = xpool.tile([128, CHUNK, E], f32)
            nc.sync.dma_start(xt[:], xv[:, ci * CHUNK:(ci + 1) * CHUNK, :])
            for j in range(CHUNK):
                # in-place: u_t overwrites the x slot it consumed
                nc.vector._custom_dve(
                    lif_op, out=xt[:, j, :], in0=prev, in1=xt[:, j, :], s0=1.0, s1=0.5
                )
                prev = xt[:, j, :]
            st = spool.tile([128, CHUNK, E], f32)
            nc.vector.tensor_scalar(
                st[:, :, :], xt[:, :, :], 1.0, None, mybir.AluOpType.is_ge
            )
            nc.sync.dma_start(ov[:, ci * CHUNK:(ci + 1) * CHUNK, :], st[:])

    nc.compile()
    _CACHE["nc"] = nc
    return nc


def _setup_axon_trace_hook():
    """Make trace=True work: inject antenv.axon_hooks + ctypes NTFF hook,
    and neuter the S3 artifact upload. Returns True on success."""
    if _CACHE.get("trace_hook_ok") is not None:
        return _CACHE["trace_hook_ok"]
    ok = False
    try:
        import importlib.util
        import types

        import antenv
        from concourse import bass_utils as bu

        if not hasattr(antenv, "axon_hooks"):
            mod = types.ModuleType("antenv.axon_hooks")
            mod._hook = None

            def set_axon_ntff_profile_hook(h):
                mod._hook = h

            def get_axon_ntff_profile_hook():
                return mod._hook

            mod.set_axon_ntff_profile_hook = set_axon_ntff_profile_hook
            mod.get_axon_ntff_profile_hook = get_axon_ntff_profile_hook
            sys.modules["antenv.axon_hooks"] = mod
            antenv.axon_hooks = mod

        spec = importlib.util.spec_from_file_location(
            "_trn_boot", "/root/.axon_site/trn_agent_boot/trn_boot.py"
        )
        tb = importlib.util.module_from_spec(spec)
        spec.loader.exec_module(tb)
        hook = tb._ntff_profile_via_ctypes("/opt/axon/libaxon_pjrt.so")
        if hook is not None:
            sys.modules["antenv.axon_hooks"].set_axon_ntff_profile_hook(hook)
            bu.upload_artifacts = lambda tmpdir: f"local://{tmpdir}"
            ok = True
    except Exception as e:  # noqa: BLE001
        print(f"trace hook setup failed: {e}", file=sys.stderr)
    _CACHE["trace_hook_ok"] = ok
    return ok


def kernel(x, threshold=None, **_ignored):
    """Full [64,200,4096] f32 in -> full spikes [64,200,4096] f32 out."""
    global LAST_EXEC_NS
    from concourse.bass_utils import run_bass_kernel_spmd

    x = np.asarray(x, dtype=np.float32)
    assert x.shape == (B, T, N), x.shape

    nc = _build()
    # host-side shard + layout transform: [8 cores][b=8, t, k=16, e] -> [128, T, E]
    xr = x.reshape(N_CORES, BS, T, K, E).transpose(0, 1, 3, 2, 4)
    in_maps = [
        {"x": np.ascontiguousarray(xr[i]).reshape(128, T, E)} for i in range(N_CORES)
    ]
    trace = bool(int(os.environ.get("BASS_LIF_TRACE", "0")))
    if trace:
        trace = _setup_axon_trace_hook()
    try:
        res = run_bass_kernel_spmd(
            nc, in_maps, core_ids=list(range(N_CORES)), trace=trace
        )
    except Exception:
        if not trace:
            raise
        res = run_bass_kernel_spmd(
            nc, in_maps, core_ids=list(range(N_CORES)), trace=False
        )
    LAST_EXEC_NS = res.exec_time_ns
    # inverse transform: [128, T, E] -> [b, t, n]
    full = np.empty((B, T, N), dtype=np.float32)
    for i in range(N_CORES):
        o = np.asarray(res.results[i]["out"]).reshape(BS, K, T, E)
        full[i * BS:(i + 1) * BS] = (
            o.transpose(0, 2, 1, 3).reshape(BS, T, N).astype(np.float32, copy=False)
        )
    return full


if __name__ == "__main__":
    rng = np.random.default_rng(0)
    xt = rng.standard_normal((B, T, N), dtype=np.float32)
    y = kernel(xt)
    print("out", y.shape, y.dtype, "mean", y.mean(), "exec_ns", LAST_EXEC_NS)
